# revision 1
# baseline (speedup 1.0000x reference)
"""DLRM-style embedding lookup kernel for 8 TRN2 NeuronCores.

Strategy: 4 table-shards x 2 row-shards. Each core holds 5 bf16 tables
(1 varlen + 4 sparse) in SBUF, vocab spread over 8 partition-groups x 16
partitions (u32-packed bf16 pairs). Host buckets every lookup into a
(row, group) slot grid (k=8 slots, 2-choice balancing via a replicated
varlen table with an independent vocab map). Device: ap_gather data +
ap_gather one-hot mask -> DVE multiply -> 16 accumulating ones-matmuls
-> per-row pooled sums in PSUM. Dense matvec folded into the same PSUM
accumulation. Host sums the 4 table-shard partials per row.
"""
import sys
sys.path.insert(0, "/opt/trn_rl_repo")

import numpy as np
import ml_dtypes
from contextlib import ExitStack

B = 65536
N_SPARSE = 16
N_VARLEN = 4
MAXLEN = 50
N_DENSE = 13
VOCAB = 1_000_000

RC = B // 2          # rows per core (2 row-shards)
K_SLOT = 8           # slots per (row, group) cell
NG = 8               # partition groups
R_TILE = 512         # rows per device tile
N_TILE = R_TILE * K_SLOT          # gather idxs per group per tile
N_TILES = RC // R_TILE            # 64
STREAM = RC * K_SLOT              # 262144 per group
NE_TBL = 6 * 4096                 # u32 pairs per partition (6 table slots)
PAD_CODE = 32

_compiled = None
TRACE = False          # set by test harness for profiled runs
_last_res = None


def _build_nc():
    import concourse.bass as bass
    import concourse.tile as tile
    from concourse import bacc, mybir

    nc = bacc.Bacc("TRN2", target_bir_lowering=False, debug=False)
    dt = mybir.dt
    tbl = nc.dram_tensor("tbl", [128, NE_TBL], dt.uint32, kind="ExternalInput").ap()
    didx = nc.dram_tensor("didx", [128, STREAM // 16], dt.int16, kind="ExternalInput").ap()
    mcode = nc.dram_tensor("mcode", [8, 2 * STREAM], dt.bfloat16, kind="ExternalInput").ap()
    expd = nc.dram_tensor("expd", [8, 128], dt.bfloat16, kind="ExternalInput").ap()
    cmpc = nc.dram_tensor("cmpc", [128, 512], dt.float32, kind="ExternalInput").ap()
    dlhs = nc.dram_tensor("dlhs", [16, 1], dt.bfloat16, kind="ExternalInput").ap()
    drhs = nc.dram_tensor("drhs", [16, RC], dt.bfloat16, kind="ExternalInput").ap()
    out = nc.dram_tensor("out", [N_TILES, R_TILE], dt.float32, kind="ExternalOutput").ap()

    with tile.TileContext(nc) as tc:
        with ExitStack() as ctx:
            pconst = ctx.enter_context(tc.tile_pool(name="const", bufs=1))
            pidx = ctx.enter_context(tc.tile_pool(name="idx", bufs=2))
            pgat = ctx.enter_context(tc.tile_pool(name="gat", bufs=2))
            pmsk = ctx.enter_context(tc.tile_pool(name="msk", bufs=2))
            pmskd = ctx.enter_context(tc.tile_pool(name="mskd", bufs=1))
            pout = ctx.enter_context(tc.tile_pool(name="out", bufs=2))
            pdense = ctx.enter_context(tc.tile_pool(name="dense", bufs=2))
            pmaskc = ctx.enter_context(tc.tile_pool(name="maskc", bufs=2))
            psum = ctx.enter_context(tc.tile_pool(name="ps", bufs=2, space="PSUM"))
            psum_bc = ctx.enter_context(tc.tile_pool(name="psbc", bufs=2, space="PSUM"))

            t_tbl = pconst.tile([128, NE_TBL], dt.uint32)
            nc.sync.dma_start(t_tbl[:], tbl[:])
            t_ones = pconst.tile([128, 1], dt.bfloat16)
            nc.vector.memset(t_ones[:], 1.0)
            t_expd = pconst.tile([8, 128], dt.bfloat16)
            nc.sync.dma_start(t_expd[:], expd[:])
            t_cmpc = pconst.tile([128, 512], dt.float32)
            nc.sync.dma_start(t_cmpc[:], cmpc[:])
            t_dlhs = pconst.tile([16, 1], dt.bfloat16)
            nc.sync.dma_start(t_dlhs[:], dlhs[:])

            for t in range(N_TILES):
                c0 = t * (N_TILE // 16)
                t_didx = pidx.tile([128, N_TILE // 16], dt.int16, tag="didx")
                nc.sync.dma_start(t_didx[:], didx[:, c0:c0 + N_TILE // 16])
                t_mcode = pmsk.tile([8, 2 * N_TILE], dt.bfloat16)
                nc.sync.dma_start(t_mcode[:], mcode[:, 2 * t * N_TILE:2 * (t + 1) * N_TILE])
                t_drhs = pdense.tile([16, R_TILE], dt.bfloat16)
                nc.sync.dma_start(t_drhs[:], drhs[:, t * R_TILE:(t + 1) * R_TILE])

                t_gat = pgat.tile([128, N_TILE, 1], dt.uint32)
                nc.gpsimd.ap_gather(t_gat[:], t_tbl[:].unsqueeze(-1), t_didx[:],
                                    channels=128, num_elems=NE_TBL, d=1, num_idxs=N_TILE)

                t_mskd = pmskd.tile([128, 2 * N_TILE], dt.bfloat16)
                gat_bf = t_gat[:].squeeze(-1).bitcast(dt.bfloat16)
                for c in range(2 * N_TILE // 512):
                    t_bc = psum_bc.tile([128, 512], dt.float32, tag="bc")
                    nc.tensor.matmul(t_bc[:], t_expd[:],
                                     t_mcode[:, c * 512:(c + 1) * 512],
                                     start=True, stop=True)
                    t_mask = pmaskc.tile([128, 512], dt.bfloat16, tag="mk")
                    nc.vector.tensor_tensor(t_mask[:], t_bc[:], t_cmpc[:],
                                            op=mybir.AluOpType.is_equal)
                    nc.vector.tensor_mul(t_mskd[:, c * 512:(c + 1) * 512],
                                         gat_bf[:, c * 512:(c + 1) * 512], t_mask[:])

                t_ps = psum.tile([1, R_TILE], dt.float32)
                nc.tensor.matmul(t_ps[:], t_dlhs[:], t_drhs[:], start=True, stop=False)
                view = t_mskd[:].rearrange("p (r x) -> p r x", x=2 * K_SLOT)
                for u in range(2 * K_SLOT):
                    nc.tensor.matmul(t_ps[:], t_ones[:], view[:, :, u],
                                     start=False, stop=(u == 2 * K_SLOT - 1))

                t_out = pout.tile([1, R_TILE], dt.float32)
                nc.scalar.copy(t_out[:], t_ps[:])
                nc.sync.dma_start(out[t:t + 1, :], t_out[:])
    nc.compile()
    return nc


def _pack_u32(bf):  # bf16 [..., 2n] -> u32 [..., n]
    u = np.ascontiguousarray(bf).view(np.uint16).astype(np.uint32)
    return (u[..., 0::2] | (u[..., 1::2] << 16)).astype(np.uint32)


def _table_images(emb_varlen_k, emb_sparse_k):
    """SBUF image [128, NE_TBL] u32 for one table-shard."""
    bf = ml_dtypes.bfloat16

    def pad(t):
        return np.concatenate([t, np.zeros(2**20 - VOCAB, t.dtype)])

    emb_varlen_k = pad(emb_varlen_k)
    emb_sparse_k = [pad(s) for s in emb_sparse_k]
    slots = []
    v = emb_varlen_k.astype(bf)
    slots.append(v.reshape(8, 16, 8192).reshape(128, 8192))                   # map0
    slots.append(v.reshape(8192, 16, 8).transpose(2, 1, 0).reshape(128, 8192))  # map1
    for s in emb_sparse_k:
        slots.append(s.astype(bf).reshape(8, 16, 8192).reshape(128, 8192))    # map0
    img = np.concatenate(slots, axis=1)            # [128, 6*8192] bf16
    return _pack_u32(img)


def _mask_table():
    one = np.float32(1.0).astype(ml_dtypes.bfloat16).view(np.uint16).astype(np.uint32)
    m = np.zeros((128, 33), np.uint32)
    p = np.arange(128) % 16
    for c in range(32):
        q, par = c // 2, c % 2
        m[p == q, c] = one << (16 * par)
    return m


def _bucket_core(vl_ids, sp_ids):
    """vl_ids [RC, 50] varlen ids; sp_ids [RC, 4] sparse ids (table order
    matches slots 2..5). Returns didx [8, RC, K], code [8, RC, K],
    fixup (rows, tslot, ids) for overflow."""
    R = vl_ids.shape[0]
    D = np.zeros((NG, R, K_SLOT), np.int16)
    C = np.full((NG, R, K_SLOT), PAD_CODE, np.int16)
    cnt = np.zeros((R, NG), np.int32)
    rows = np.arange(R)
    fix_r, fix_t, fix_i = [], [], []

    def place(g, epair, code, valid):
        s = cnt[rows, g]
        ok = valid & (s < K_SLOT)
        D[g[ok], rows[ok], s[ok]] = epair[ok]
        C[g[ok], rows[ok], s[ok]] = code[ok]
        cnt[rows[ok], g[ok]] += 1
        return valid & ~ok

    for j in range(4):
        ids = sp_ids[:, j].astype(np.int64)
        g = (ids >> 17).astype(np.int32) & 7
        epair = ((2 + j) * 4096 + ((ids >> 1) & 4095)).astype(np.int16)
        code = (2 * ((ids >> 13) & 15) + (ids & 1)).astype(np.int16)
        ovf = place(g, epair, code, np.ones(R, bool))
        for r in np.nonzero(ovf)[0]:
            fix_r.append(r); fix_t.append(2 + j); fix_i.append(ids[r])

    for j in range(MAXLEN):
        ids = vl_ids[:, j].astype(np.int64)
        valid = ids > 0
        g0 = (ids >> 17).astype(np.int32) & 7
        e0 = (0 * 4096 + ((ids >> 1) & 4095)).astype(np.int16)
        c0 = (2 * ((ids >> 13) & 15) + (ids & 1)).astype(np.int16)
        g1 = (ids & 7).astype(np.int32)
        e1 = (1 * 4096 + (ids >> 8)).astype(np.int16)
        c1 = (2 * ((ids >> 3) & 15) + ((ids >> 7) & 1)).astype(np.int16)
        use0 = cnt[rows, g0] <= cnt[rows, g1]
        g_a = np.where(use0, g0, g1); e_a = np.where(use0, e0, e1); c_a = np.where(use0, c0, c1)
        g_b = np.where(use0, g1, g0); e_b = np.where(use0, e1, e0); c_b = np.where(use0, c1, c0)
        ovf = place(g_a, e_a, c_a, valid)
        ovf = place(g_b, e_b, c_b, ovf)
        for r in np.nonzero(ovf)[0]:
            fix_r.append(r); fix_t.append(0); fix_i.append(ids[r])
    return D, C, (np.array(fix_r, np.int64), np.array(fix_t, np.int64),
                  np.array(fix_i, np.int64))


def _wrap(stream_g):  # [NG, STREAM] -> [128, STREAM//16] int16
    out = np.empty((128, STREAM // 16), np.int16)
    for g in range(NG):
        out[16 * g:16 * g + 16] = stream_g[g].reshape(STREAM // 16, 16).T
    return out


def prepare_in_maps(sparse_ids, varlen_ids, dense_vals, emb_sparse, emb_varlen,
                    dense_weight):
    """Host sharding: returns (in_maps for 8 cores, fixup vector [B])."""
    sparse_ids = np.asarray(sparse_ids); varlen_ids = np.asarray(varlen_ids)
    dense_vals = np.asarray(dense_vals, np.float32)
    emb_sparse = np.asarray(emb_sparse, np.float32)
    emb_varlen = np.asarray(emb_varlen, np.float32)
    dense_weight = np.asarray(dense_weight, np.float32)
    bf = ml_dtypes.bfloat16

    dw = np.zeros((16, 1), np.float32); dw[:N_DENSE] = dense_weight
    dv_t = np.zeros((16, B), np.float32); dv_t[:N_DENSE] = dense_vals.T
    # blockdiag expander: group-row g -> partitions [16g, 16g+16)
    expd = np.zeros((8, 128), np.float32)
    for g in range(NG):
        expd[g, 16 * g:16 * g + 16] = 1.0
    # compare constant: CONST[p, x] = 2*(p%16) + (x&1)
    cmpc = (2 * (np.arange(128) % 16)[:, None]
            + (np.arange(512) & 1)[None, :]).astype(np.float32)

    in_maps = []
    fixups = np.zeros(B, np.float64)
    for c in range(8):
        k, h = c // 2, c % 2
        r0 = h * RC
        tbl_img = _table_images(emb_varlen[k], emb_sparse[4 * k:4 * k + 4])
        vl = varlen_ids[r0:r0 + RC, k, :]
        sp = sparse_ids[r0:r0 + RC, 4 * k:4 * k + 4]
        D, C, (fr, ft, fi) = _bucket_core(vl, sp)
        if len(fr):
            tabs = np.stack([emb_varlen[k], emb_varlen[k],
                             emb_sparse[4 * k], emb_sparse[4 * k + 1],
                             emb_sparse[4 * k + 2], emb_sparse[4 * k + 3]])
            np.add.at(fixups, r0 + fr, tabs[ft, fi].astype(bf).astype(np.float64))
        dstream = D.reshape(NG, STREAM)
        cstream = C.reshape(NG, STREAM).astype(np.float32)
        cstream = np.where(cstream >= 32, -1.0, cstream)      # pads match nothing
        mcode = np.repeat(cstream, 2, axis=1).astype(bf)      # [8, 2*STREAM]
        in_maps.append(dict(
            tbl=tbl_img, didx=_wrap(dstream), mcode=mcode,
            expd=expd.astype(bf), cmpc=cmpc,
            dlhs=(dw if k == 0 else np.zeros_like(dw)).astype(bf),
            drhs=dv_t[:, r0:r0 + RC].astype(bf)))
    return in_maps, fixups


def assemble_output(results, fixups):
    out = np.zeros(B, np.float64)
    for c in range(8):
        k, h = c // 2, c % 2
        out[h * RC:(h + 1) * RC] += results[c]["out"].reshape(RC).astype(np.float64)
    out += fixups
    return out.astype(np.float32).reshape(B, 1)


def kernel(sparse_ids, varlen_ids, dense_vals, emb_sparse, emb_varlen, dense_weight):
    global _compiled
    from concourse import bass_utils

    in_maps, fixups = prepare_in_maps(sparse_ids, varlen_ids, dense_vals,
                                      emb_sparse, emb_varlen, dense_weight)
    if _compiled is None:
        _compiled = _build_nc()
    res = bass_utils.run_bass_kernel_spmd(_compiled, in_maps,
                                          core_ids=list(range(8)))
    global _last_res
    _last_res = res
    return assemble_output(res.results, fixups)



# revision 2
# speedup vs baseline: 1.0641x; 1.0641x over previous
"""DLRM-style 1-d embedding lookup via indirect-DMA element gather, 8 TRN2 cores.

Sharding: 4 table-groups x 2 row-halves. Core (k, h) owns rows
[h*32768, (h+1)*32768) and tables {varlen k, sparse 4k..4k+3} stored flat in
DRAM as f32 [5*VOCAB + 16] (entry 5*VOCAB = 0.0 catches varlen pads).

indirect_dma_start semantics (measured on hw): with dest AP [1, C, 1] it
generates C single-element descriptors, all writing the dest AP's partition
row sequentially; the n-th descriptor's offset is read from the offset AP at
[n % 128, n // 128]. So: one instruction per partition p gathers that
partition's whole stream (RPP rows x 54 lookups) into t_g[p, :]. One DVE
tensor_reduce then sums each row's 54 values. Dense matmul + cross-core sum
happen on host in f32 (exact).

Row -> (partition, column) map: local row q -> partition q // RPP, j = q % RPP.
"""
import sys
sys.path.insert(0, "/opt/trn_rl_repo")

import numpy as np

B = 65536
N_SPARSE = 16
N_VARLEN = 4
MAXLEN = 50
N_DENSE = 13
VOCAB = 1_000_000

RC = B // 2              # rows per core
RPP = RC // 128          # rows per partition (256)
K = MAXLEN + 4           # lookups per row (54)
C = RPP * K              # gathered f32 per partition (13824)
CW = C // 128            # offset columns per partition-instruction (108)
ZERO_OFF = 5 * VOCAB     # offset of the 0.0 entry
TBL_LEN = 5 * VOCAB + 16

_compiled = None
_last_res = None


def _build_nc():
    import concourse.bass as bass
    import concourse.tile as tile
    from concourse import bacc, mybir
    from contextlib import ExitStack

    nc = bacc.Bacc("TRN2", target_bir_lowering=False, debug=False)
    dt = mybir.dt
    tbl = nc.dram_tensor("tbl", [TBL_LEN, 1], dt.float32, kind="ExternalInput").ap()
    offs = nc.dram_tensor("offs", [128, 128 * CW], dt.int32, kind="ExternalInput").ap()
    out = nc.dram_tensor("out", [128, RPP], dt.float32, kind="ExternalOutput").ap()

    with tile.TileContext(nc) as tc:
        with ExitStack() as ctx:
            pbuf = ctx.enter_context(tc.tile_pool(name="buf", bufs=1))

            t_off = pbuf.tile([128, 128 * CW], dt.int32)
            nc.sync.dma_start(t_off[:], offs[:])
            t_g = pbuf.tile([128, C], dt.float32)
            for p in range(128):
                nc.gpsimd.indirect_dma_start(
                    out=t_g[p:p + 1, :].unsqueeze(-1),
                    out_offset=None,
                    in_=tbl[:],
                    in_offset=bass.IndirectOffsetOnAxis(
                        ap=t_off[:, p * CW:(p + 1) * CW], axis=0),
                )
            t_r = pbuf.tile([128, RPP], dt.float32)
            nc.vector.tensor_reduce(
                out=t_r[:], in_=t_g[:].rearrange("p (r k) -> p r k", k=K),
                axis=mybir.AxisListType.X, op=mybir.AluOpType.add)
            nc.sync.dma_start(out[:], t_r[:])
    nc.compile()
    return nc


def prepare_in_maps(sparse_ids, varlen_ids, dense_vals, emb_sparse, emb_varlen,
                    dense_weight):
    sparse_ids = np.asarray(sparse_ids)
    varlen_ids = np.asarray(varlen_ids)
    emb_sparse = np.asarray(emb_sparse, np.float32)
    emb_varlen = np.asarray(emb_varlen, np.float32)

    in_maps = []
    for c in range(8):
        k, h = c // 2, c % 2
        rows = slice(h * RC, (h + 1) * RC)
        tblv = np.empty((TBL_LEN, 1), np.float32)
        tblv[:VOCAB, 0] = emb_varlen[k]
        for j in range(4):
            tblv[(1 + j) * VOCAB:(2 + j) * VOCAB, 0] = emb_sparse[4 * k + j]
        tblv[5 * VOCAB:, 0] = 0.0

        vl = varlen_ids[rows, k, :]                           # [RC, 50] int32
        vl = np.where(vl > 0, vl, ZERO_OFF)
        sp = sparse_ids[rows, 4 * k:4 * k + 4].astype(np.int64)
        sp = sp + (np.arange(1, 5, dtype=np.int64) * VOCAB)[None, :]
        off = np.concatenate([vl.astype(np.int64), sp], axis=1).astype(np.int32)
        # off[q, k]: local row q = p*RPP + j.  Partition p's stream (dest
        # order) is n = j*K + k; instruction p reads its n-th offset at
        # [n % 128, p*CW + n // 128].
        stream = off.reshape(128, C)                 # [p, n]
        arr = stream.reshape(128, CW, 128)           # [p, c, r]
        offs_host = np.ascontiguousarray(
            arr.transpose(2, 0, 1).reshape(128, 128 * CW))
        in_maps.append(dict(tbl=tblv, offs=offs_host))
    return in_maps


def assemble_output(results, dense_vals, dense_weight):
    dense_vals = np.asarray(dense_vals, np.float32)
    dense_weight = np.asarray(dense_weight, np.float32)
    out = np.zeros(B, np.float32)
    for c in range(8):
        h = c % 2
        out[h * RC:(h + 1) * RC] += results[c]["out"].reshape(RC)
    return out[:, None] + dense_vals @ dense_weight


def kernel(sparse_ids, varlen_ids, dense_vals, emb_sparse, emb_varlen, dense_weight):
    global _compiled, _last_res
    from concourse import bass_utils

    in_maps = prepare_in_maps(sparse_ids, varlen_ids, dense_vals,
                              emb_sparse, emb_varlen, dense_weight)
    if _compiled is None:
        _compiled = _build_nc()
    res = bass_utils.run_bass_kernel_spmd(_compiled, in_maps,
                                          core_ids=list(range(8)))
    _last_res = res
    return assemble_output(res.results, dense_vals, dense_weight)


# revision 3
# speedup vs baseline: 1.0830x; 1.0178x over previous
"""DLRM-style 1-d embedding lookup via indirect-DMA element gather, 8 TRN2 cores.

Sharding: 4 table-groups x 2 row-halves. Core (k, h) owns rows
[h*32768, (h+1)*32768) and tables {varlen k, sparse 4k..4k+3} stored flat in
DRAM as f32 [5*VOCAB + 16] (entry 5*VOCAB = 0.0 catches varlen pads).

indirect_dma_start semantics (measured on hw): with dest AP [1, C, 1] it
generates C single-element descriptors, all writing the dest AP's partition
row sequentially; the n-th descriptor's offset is read from the offset AP at
[n % 128, n // 128]. So: one instruction per partition p gathers that
partition's whole stream (RPP rows x 54 lookups) into t_g[p, :]. One DVE
tensor_reduce then sums each row's 54 values. Dense matmul + cross-core sum
happen on host in f32 (exact).

Row -> (partition, column) map: local row q -> partition q // RPP, j = q % RPP.
"""
import sys
sys.path.insert(0, "/opt/trn_rl_repo")

import numpy as np

B = 65536
N_SPARSE = 16
N_VARLEN = 4
MAXLEN = 50
N_DENSE = 13
VOCAB = 1_000_000

RC = B // 2              # rows per core
RPP = RC // 128          # rows per partition (256)
K = MAXLEN + 4           # lookups per row (54)
C = RPP * K              # gathered f32 per partition (13824)
CW = C // 128            # offset columns per partition-instruction (108)
ZERO_OFF = 5 * VOCAB     # offset of the 0.0 entry
TBL_LEN = 5 * VOCAB + 16

_compiled = None
_last_res = None


def _build_nc():
    import concourse.bass as bass
    import concourse.tile as tile
    from concourse import bacc, mybir
    from contextlib import ExitStack

    nc = bacc.Bacc("TRN2", target_bir_lowering=False, debug=False)
    dt = mybir.dt
    tbl = nc.dram_tensor("tbl", [TBL_LEN, 1], dt.float32, kind="ExternalInput").ap()
    offs = nc.dram_tensor("offs", [128, 128 * CW], dt.int32, kind="ExternalInput").ap()
    out = nc.dram_tensor("out", [128, RPP], dt.float32, kind="ExternalOutput").ap()

    with tile.TileContext(nc) as tc:
        with ExitStack() as ctx:
            pbuf = ctx.enter_context(tc.tile_pool(name="buf", bufs=1))

            t_off = pbuf.tile([128, 128 * CW], dt.int32)
            nc.sync.dma_start(t_off[:], offs[:])
            t_g = pbuf.tile([128, C], dt.float32)
            for p in range(128):
                inst = nc.gpsimd.indirect_dma_start(
                    out=t_g[p:p + 1, :].unsqueeze(-1),
                    out_offset=None,
                    in_=tbl[:],
                    in_offset=bass.IndirectOffsetOnAxis(
                        ap=t_off[:, p * CW:(p + 1) * CW], axis=0),
                )
                inst.ins.single_packet = False
            t_r = pbuf.tile([128, RPP], dt.float32)
            nc.vector.tensor_reduce(
                out=t_r[:], in_=t_g[:].rearrange("p (r k) -> p r k", k=K),
                axis=mybir.AxisListType.X, op=mybir.AluOpType.add)
            nc.sync.dma_start(out[:], t_r[:])
    nc.compile()
    return nc


def prepare_in_maps(sparse_ids, varlen_ids, dense_vals, emb_sparse, emb_varlen,
                    dense_weight):
    sparse_ids = np.asarray(sparse_ids)
    varlen_ids = np.asarray(varlen_ids)
    emb_sparse = np.asarray(emb_sparse, np.float32)
    emb_varlen = np.asarray(emb_varlen, np.float32)

    in_maps = []
    for c in range(8):
        k, h = c // 2, c % 2
        rows = slice(h * RC, (h + 1) * RC)
        tblv = np.empty((TBL_LEN, 1), np.float32)
        tblv[:VOCAB, 0] = emb_varlen[k]
        for j in range(4):
            tblv[(1 + j) * VOCAB:(2 + j) * VOCAB, 0] = emb_sparse[4 * k + j]
        tblv[5 * VOCAB:, 0] = 0.0

        vl = varlen_ids[rows, k, :]                           # [RC, 50] int32
        vl = np.where(vl > 0, vl, ZERO_OFF)
        sp = sparse_ids[rows, 4 * k:4 * k + 4].astype(np.int64)
        sp = sp + (np.arange(1, 5, dtype=np.int64) * VOCAB)[None, :]
        off = np.concatenate([vl.astype(np.int64), sp], axis=1).astype(np.int32)
        # off[q, k]: local row q = p*RPP + j.  Partition p's stream (dest
        # order) is n = j*K + k; instruction p reads its n-th offset at
        # [n % 128, p*CW + n // 128].
        stream = off.reshape(128, C)                 # [p, n]
        arr = stream.reshape(128, CW, 128)           # [p, c, r]
        offs_host = np.ascontiguousarray(
            arr.transpose(2, 0, 1).reshape(128, 128 * CW))
        in_maps.append(dict(tbl=tblv, offs=offs_host))
    return in_maps


def assemble_output(results, dense_vals, dense_weight):
    dense_vals = np.asarray(dense_vals, np.float32)
    dense_weight = np.asarray(dense_weight, np.float32)
    out = np.zeros(B, np.float32)
    for c in range(8):
        h = c % 2
        out[h * RC:(h + 1) * RC] += results[c]["out"].reshape(RC)
    return out[:, None] + dense_vals @ dense_weight


def kernel(sparse_ids, varlen_ids, dense_vals, emb_sparse, emb_varlen, dense_weight):
    global _compiled, _last_res
    from concourse import bass_utils

    in_maps = prepare_in_maps(sparse_ids, varlen_ids, dense_vals,
                              emb_sparse, emb_varlen, dense_weight)
    if _compiled is None:
        _compiled = _build_nc()
    res = bass_utils.run_bass_kernel_spmd(_compiled, in_maps,
                                          core_ids=list(range(8)))
    _last_res = res
    return assemble_output(res.results, dense_vals, dense_weight)
